# revision 21
# baseline (speedup 1.0000x reference)
"""2-layer GAT on 8 Trainium2 NeuronCores (Bass/Tile).

Strategy (dst-partitioned, gather-based):
- Nodes are partitioned contiguously across 8 cores by destination; each core
  handles all edges whose dst lands in its range, so per-core outputs and the
  per-destination softmax segments are fully local (no cross-core reduction).
- Per layer, each core computes node rows [z | es | ed] for its own nodes with
  TensorE matmuls, the 8 slices are AllGather-ed into a replicated DRAM table,
  and each core uses `dma_gather` (512B rows) to fetch z/es of every edge's
  source node.
- Edges are laid out host-side in a (node-partition x slot) grid: each 128-node
  chunk gets T slot-tiles; tile t holds the t-th incoming edge of each node in
  partition p. Nodes are bucketed by in-degree (split into low/high source
  ranges for int16 gather indices, superblock-sorted on both counts) so
  padding is small. Pad slots point at a dummy table row with es = -1e30,
  which exp() maps to an exact 0 weight.
- Per slot-tile: ex = exp(leaky_relu(es_src + ed_dst)) on DVE/ACT written into
  a fused [pay | ex] bf16 tile, payload ex*z on DVE, then ONE
  identity-stationary matmul per tile accumulates the weighted sum and the
  softmax denominator together into PSUM. A per-chunk epilogue divides,
  applies elu (layer 1), and computes the next layer's node rows.
- Layer-1 table rows are 512B (z bf16 + es/ed f32); layer-2 rows are 256B
  (z2 bf16 + es2/ed2 f32), halving the second AllGather. Tables are Shared
  DRAM (fast collective path). Groups are processed largest-first so the
  drain into each AllGather is short. dma_gather is q7 desc-gen bound
  (~7.8 ns/row); prep/trigger splitting and >1024-idx calls do NOT help.

Landed 2026-08-10 (2.31ms -> 2.00ms, rel_err unchanged 0.00629):
- SIGNED-IDX SINGLE GRID (build_plan_sg, GAT_SG=1 default): the q7 gather
  address math is IVP_MULUSAN (unsigned stride x SIGNED idx), HW-verified
  with bench_neg.py - so idx int16 = pos - 32768 with the gather AP based
  at table row 32768 reaches all 50184 rows. Kills the lo/hi range split:
  one degree sort -> 813 tiles vs 924 (-12% gather chain, -314us).
  Constraint: the q7 TRIMS a trailing run of NEGATIVE idx per call, so the
  last flat slot (call-final tile, p127) of every gather call must hold a
  value >= 32768 or the dummy; enforced by reordering the p127 node's own
  edges among its slots (order-invariant math) with node swaps as
  fallback; build asserts verify every call-final element.
- AG PACKING (GAT_AGPACK, default OFF): moving only the meaningful row
  bytes via strided collective APs (table1: 72 of 128 f32/row) passes the
  Tile sim but CRASHES the axon/PJRT runtime (JaxRuntimeError INTERNAL at
  execute) - the CC lowering does not accept 3-dim strided views. Keep 0.
- Shared-DRAM tensors allow only ONE writer instruction: a table cannot
  be filled by two split AllGathers (tried for compute/AG overlap; the
  Tile event loop rejects it). Split-AG would need per-half tables plus a
  gather source spanning both - not expressible as one AP.

Perf findings (trace-verified, 2026-08-10 session; HW 2.31ms):
- DMAGatherAnt on GpSimd = 2.04ms busy (88% of span), 276 calls x ~8.6us
  per 1024 idx = 8.4 ns/idx, uniform (no per-call fixed cost: a 537-idx
  call took 4.6us). DMA engines only ~45% busy (~53ns/desc = 3.3ns/row
  wall over 16 engines). Everything else (PE 530us, DVE 833us, 2 AGs
  160us) overlaps under the gather serial chain.
- dma_gather executes on ONE q7 core pair picked by queue_num
  (dma_gather.cpp: cpu_id/2 == queue_num); the Pool engine serializes
  instructions, so multi-queue calls do NOT overlap desc-gen. A custom
  4-queue q7 kernel would fix it, but the container has NO xtensa
  toolchain (library_overlay.build_library needs nix ucodeEnvWithTools;
  prebuilt-deps tarballs are LFS stubs) - dead end here.
- The cost model's 0.34ns/desc is the CounterMachine dma_start path,
  not dma_gather's software desc loop (scalar idx unpack + per-16-lane
  pushes) - don't trust it for gathers.
- indirect_dma_start (InstDMACopy + DynamicAP, bench_gather.py):
  desc-gen is 0.56ns/row (INDIRECT1D 2.3us per 4096 rows - 15x better
  than dma_gather!) BUT (a) with a contiguous [P,T,128] f32 dest the
  lowering MERGES rows into 1024B descs consuming one index per TWO rows
  (wrong data); (b) with non-contiguous rows ([P,T,132] view 0:128) the
  gathered rows didn't match the index set at all (mapping unresolved);
  (c) DMA transfer is ~330ns/desc = 20.7ns/row over 16 engines (engines
  99% busy, un-concatenated packets) vs dma_gather's 3.3ns/row
  single_packet stream. A ~20% tile offload hybrid (isplits/idx32_* in
  the plan are groundwork) would only pay if (b) gets solved - derive
  the HW index-consumption order first with bench_gather.py's probe.
- GAT_SB=12 is the tile-count optimum (924); global degree-balanced
  round-robin node->core assignment makes padding WORSE (1407 tiles).
- ap_gather/gather_transpose (free-dim SBUF gather) are the wrong layout
  (edges land on free dim; PE transposes to fix cost ~3x the PE budget),
  and likely ~100cyc/4idx on cayman (ReadOverlap=0 pitfall).
"""
import sys

sys.path.insert(0, "/opt/trn_rl_repo")

import numpy as np
import ml_dtypes

import concourse.bass as bass
import concourse.bacc as bacc
import concourse.mybir as mybir
import concourse.tile as tile
from concourse.library_config import mlp

F32 = mybir.dt.float32
BF16 = mybir.dt.bfloat16
I16 = mybir.dt.int16
AF = mybir.ActivationFunctionType
ALU = mybir.AluOpType
BF = ml_dtypes.bfloat16

C = 8          # cores
P = 128        # partitions


# --------------------------------------------------------------------------
# host-side preprocessing
# --------------------------------------------------------------------------

class Plan:
    """Host-computed layout shared by the program builder and per-core data."""


def build_plan(src, dst, n_nodes, group_chunks=None):
    import os
    if group_chunks is None:
        group_chunks = int(os.environ.get("GAT_GROUP", "2"))
    pl = Plan()
    npc = n_nodes // C
    assert npc * C == n_nodes
    chunks = -(-npc // P)
    npad = chunks * P
    slice_n = npad + 1              # + dummy row
    tbl_n = C * slice_n
    # low/high split for int16 gather indices
    hi_core = (C + 1) // 2          # cores [0,hi_core) low, rest high
    while hi_core * slice_n > 32768:
        hi_core -= 1
    assert (C - hi_core) * slice_n <= 32768, "table too large for 2-way split"
    hi_base = hi_core * slice_n
    pl.npc, pl.chunks, pl.npad = npc, chunks, npad
    pl.slice_n, pl.tbl_n, pl.hi_core, pl.hi_base = slice_n, tbl_n, hi_core, hi_base

    owner = dst // npc
    src_owner = src // npc
    is_lo = src_owner < hi_core

    # per-core, per-node in-edge lists split by src range
    perm = np.zeros((C, npad), np.int64)        # processing order -> local id
    klo = np.zeros((C, npad), np.int32)
    khi = np.zeros((C, npad), np.int32)
    edges_lo = []                                # per core: [n_lo_edges] srcs sorted by (dstlocal)
    edges_hi = []
    sb = int(os.environ.get("GAT_SB", "12")) * P  # superblock resort size
    for c in range(C):
        m = owner == c
        d_loc = dst[m] - c * npc
        s = src[m]
        lo_m = is_lo[m]
        cnt_lo = np.bincount(d_loc[lo_m], minlength=npc)
        cnt_hi = np.bincount(d_loc[~lo_m], minlength=npc)
        order = np.lexsort((cnt_hi, cnt_lo))     # sort nodes by (klo, khi)
        if sb > 0:
            # re-sort by khi within superblocks: keeps klo nearly sorted
            # (narrow range per block) while making khi sorted within each
            # block, shrinking both per-chunk maxima.
            kh_o = cnt_hi[order]
            for b in range(0, npc, sb):
                e = min(b + sb, npc)
                sub = np.argsort(kh_o[b:e], kind="stable")
                order[b:e] = order[b:e][sub]
        perm[c, :npc] = order
        perm[c, npc:] = npc                      # phantom marker
        klo[c, :npc] = cnt_lo[order]
        khi[c, :npc] = cnt_hi[order]
        # edge lists grouped by local dst: sort edges by d_loc
        o_lo = np.argsort(d_loc[lo_m], kind="stable")
        o_hi = np.argsort(d_loc[~lo_m], kind="stable")
        edges_lo.append((d_loc[lo_m][o_lo], s[lo_m][o_lo]))
        edges_hi.append((d_loc[~lo_m][o_hi], s[~lo_m][o_hi]))

    # global per-chunk tile counts
    kl = klo.reshape(C, chunks, P)
    kh = khi.reshape(C, chunks, P)
    tlo = np.maximum(kl.max(axis=(0, 2)), 1)     # [chunks], >= 1
    thi = kh.max(axis=(0, 2))                    # [chunks]
    pl.tlo, pl.thi = tlo, thi

    # groups of chunks per gather call, processed largest-first so the
    # epilogue drain before each AllGather is a small chunk's chain
    pl.group = group_chunks
    groups = [list(range(g, min(g + group_chunks, chunks)))
              for g in range(0, chunks, group_chunks)]
    tilesum = tlo + thi
    groups.sort(key=lambda g: -int(tilesum[g].sum()))
    # split the smallest (last-processed) group into singletons: the
    # epilogue drain into each AllGather and the final tail are one small
    # chunk's chain instead of two
    if len(groups[-1]) > 1:
        last = groups.pop()
        groups.extend([[k] for k in last])
    pl.groups = groups

    # per-group tile counts routed to the indirect-DMA path (tail tiles of
    # the group's lo/hi ranges): Pool desc-gen is ~15x cheaper there, DMA
    # per-row ~6x dearer, so a small fraction balances the two.
    ifrac = float(os.environ.get("GAT_IFRAC", "0.2"))
    pl.isplits = [(int(ifrac * int(tlo[g].sum())),
                   int(ifrac * int(thi[g].sum()))) for g in groups]

    # position of original node v in the table: owner*slice_n + invperm
    invperm = np.zeros((C, npc), np.int64)
    for c in range(C):
        invperm[c, perm[c, :npc]] = np.arange(npad)[: npc]
    pos = (owner_all := np.arange(n_nodes) // npc) * slice_n \
        + invperm[owner_all, np.arange(n_nodes) % npc]
    pl.pos = pos
    pl.perm = perm

    # build per-core int16 gather index arrays (tile-major inside groups)
    dummy_rel = npad                            # dummy row, relative to base
    idx_lo = np.full((C, int(tlo.sum()) * P), dummy_rel, np.int32)
    idx_hi = np.full((C, int(thi.sum()) * P), dummy_rel, np.int32)
    lo_tile_base = np.concatenate([[0], np.cumsum(tlo)])   # per chunk
    hi_tile_base = np.concatenate([[0], np.cumsum(thi)])
    for c in range(C):
        for (d_loc, s), karr, idx, tbase, tcnt, base_off in (
            (edges_lo[c], kl[c], idx_lo[c], lo_tile_base, tlo, 0),
            (edges_hi[c], kh[c], idx_hi[c], hi_tile_base, thi, pl.hi_base),
        ):
            if len(d_loc) == 0:
                continue
            posv = pos[s] - base_off
            # slot index of each edge within its node's list (0..k-1)
            # edges are sorted by d_loc; slot = running index within node
            slot = np.arange(len(d_loc)) - np.concatenate(
                [[0], np.cumsum(np.bincount(d_loc, minlength=npc))])[d_loc]
            # node -> (chunk, partition) via invperm
            ip = invperm[c, d_loc]
            ch, p = ip // P, ip % P
            flat = (tbase[ch] + slot) * P + p
            idx[flat] = posv
    assert idx_lo.max() < 32768 and idx_hi.max() < 32768
    pl.idx_lo, pl.idx_hi = idx_lo.astype(np.int16), idx_hi.astype(np.int16)
    pl.lo_tile_base, pl.hi_tile_base = lo_tile_base, hi_tile_base

    # absolute-row i32 index grids [P, tiles] for the indirect-DMA hybrid:
    # idx32_*[p, tile] = global table row for slot (p, tile); pads -> dummy
    # (row npad inside core-0's slice).
    ntl, nth = int(tlo.sum()), int(thi.sum())
    idx32_lo = np.full((C, P, max(ntl, 1)), npad, np.int32)
    idx32_hi = np.full((C, P, max(nth, 1)), npad, np.int32)
    for c in range(C):
        f = idx_lo[c].astype(np.int64)
        m = f != dummy_rel
        flat = np.nonzero(m)[0]
        idx32_lo[c, flat % P, flat // P] = f[m]
        f = idx_hi[c].astype(np.int64)
        m = f != dummy_rel
        flat = np.nonzero(m)[0]
        idx32_hi[c, flat % P, flat // P] = f[m] + hi_base
    pl.idx32_lo, pl.idx32_hi = idx32_lo, idx32_hi
    return pl


def wrap_idx(arr):
    """[n] int16 -> [128, n/16] wrapped + replicated across the 8 q7 cores."""
    n = arr.shape[0]
    assert n % 16 == 0
    w = arr.reshape(n // 16, 16).T               # [16, n/16]
    return np.tile(w, (8, 1)).copy()


SBASE = 32768   # gather AP base row: idx int16 = pos - SBASE (signed)


def build_plan_sg(src, dst, n_nodes, group_chunks=None):
    """Single-grid plan: signed int16 gather indices (HW-verified: the q7
    address mul-acc is IVP_MULUSAN = unsigned*SIGNED, so idx = pos - 32768
    with the gather AP based at table row 32768 spans all 50184 rows).
    Kills the lo/hi split: one degree sort -> 813 tiles vs 924.

    Trailing-trim guard: the q7 drops a TRAILING run of negative idx per
    call, so the last flat slot (tile call_end, p127) of every gather call
    must hold a value >= SBASE (high row or the dummy). Enforced by
    reordering the p127 node's own edges (order-invariant math) and, where
    that can't work, swapping which node sits at p127 of the chunk.
    """
    import os
    if group_chunks is None:
        group_chunks = int(os.environ.get("GAT_GROUP", "2"))
    mt = int(os.environ.get("GAT_MAXIDX", "1024")) // P
    pl = Plan()
    npc = n_nodes // C
    assert npc * C == n_nodes
    chunks = -(-npc // P)
    npad = chunks * P
    slice_n = npad + 1
    tbl_n = C * slice_n
    pl.npc, pl.chunks, pl.npad = npc, chunks, npad
    pl.slice_n, pl.tbl_n = slice_n, tbl_n
    pl.signed_grid = True
    pl.hi_core, pl.hi_base = C, C * slice_n      # unused; keep fields valid

    owner = dst // npc
    deg = np.zeros((C, npc), np.int64)
    d_locs, srcs = [], []
    for c in range(C):
        m = owner == c
        d_loc = dst[m] - c * npc
        s = src[m]
        o = np.argsort(d_loc, kind="stable")
        d_locs.append(d_loc[o])
        srcs.append(s[o])
        deg[c] = np.bincount(d_loc, minlength=npc)

    perm = np.zeros((C, npad), np.int64)
    for c in range(C):
        perm[c, :npc] = np.argsort(deg[c], kind="stable")
        perm[c, npc:] = npc

    kcnt = np.zeros((C, npad), np.int32)
    for c in range(C):
        kcnt[c, :npc] = deg[c][perm[c, :npc]]
    kk = kcnt.reshape(C, chunks, P)
    tiles = np.maximum(kk.max(axis=(0, 2)), 1)
    tile_base = np.concatenate([[0], np.cumsum(tiles)])
    pl.tlo, pl.thi = tiles.astype(np.int64), np.zeros(chunks, np.int64)
    pl.lo_tile_base = tile_base
    pl.hi_tile_base = np.zeros(chunks + 1, np.int64)

    # groups (largest first; last multi-chunk group split to singletons)
    pl.group = group_chunks
    groups = [list(range(g, min(g + group_chunks, chunks)))
              for g in range(0, chunks, group_chunks)]
    groups.sort(key=lambda g: -int(tiles[g].sum()))
    if len(groups[-1]) > 1:
        last = groups.pop()
        groups.extend([[k] for k in last])
    pl.groups = groups
    pl.isplits = [(0, 0)] * len(groups)

    # call-final global tiles (per group range, mt tiles per gather call)
    finals = set()
    for g in groups:
        b, n = int(tile_base[g[0]]), int(tiles[g].sum())
        done = 0
        while done < n:
            nt = min(n - done, mt)
            finals.add(b + done + nt - 1)
            done += nt

    def make_pos():
        invperm = np.zeros((C, npc), np.int64)
        for c in range(C):
            invperm[c, perm[c, :npc]] = np.arange(npc)
        ow = np.arange(n_nodes) // npc
        return invperm, ow * slice_n + invperm[ow, np.arange(n_nodes) % npc]

    invperm, pos = make_pos()

    # guard feasibility: for each (core, chunk) the p127 node (rank
    # k*128+127) must own >= (#guards below its degree) edges with
    # pos[src] >= SBASE. Swap in a feasible node where needed.
    def nhigh_of(c):
        hi = (pos[srcs[c]] >= SBASE).astype(np.int64)
        return np.bincount(d_locs[c], weights=hi, minlength=npc).astype(
            np.int64)

    for it in range(3):
        swapped = 0
        for c in range(C):
            nh = nhigh_of(c)
            for k in range(chunks):
                gset = [t - int(tile_base[k]) for t in finals
                        if tile_base[k] <= t < tile_base[k] + tiles[k]]
                if not gset:
                    continue
                r = k * P + 127
                if r >= npc:
                    continue

                def ok(u):
                    return nh[u] >= sum(1 for t in gset if t < deg[c][u])

                v = perm[c, r]
                if ok(v):
                    continue
                band = perm[c, k * P:(k + 1) * P]
                cand = [j for j in range(P - 1) if ok(band[j])]
                assert cand, f"no guard-feasible node c={c} k={k}"
                j = cand[-1]
                perm[c, k * P + j], perm[c, r] = perm[c, r], perm[c, k * P + j]
                swapped += 1
        if swapped == 0:
            break
        invperm, pos = make_pos()   # swaps moved table rows; recompute
    pl.pos, pl.perm = pos, perm

    # build flat idx (absolute rows), then guard-reorder p127 nodes' slots
    dummy_global = 6 * slice_n + npad            # high slice -> idx >= 0
    assert dummy_global >= SBASE
    ntile = int(tiles.sum())
    idx = np.full((C, ntile * P), dummy_global, np.int64)
    for c in range(C):
        d_loc, s = d_locs[c], srcs[c]
        posv = pos[s]
        slot = np.arange(len(d_loc)) - np.concatenate(
            [[0], np.cumsum(np.bincount(d_loc, minlength=npc))])[d_loc]
        ip = invperm[c, d_loc]
        ch, p = ip // P, ip % P
        idx[c, (tile_base[ch] + slot) * P + p] = posv
        for k in range(chunks):
            gset = [t - int(tile_base[k]) for t in finals
                    if tile_base[k] <= t < tile_base[k] + tiles[k]]
            r = k * P + 127
            if not gset or r >= npc:
                continue
            v = perm[c, r]
            kv = int(deg[c][v])
            slots = (tile_base[k] + np.arange(kv)) * P + 127
            vals = idx[c, slots]
            need = [t for t in gset if t < kv]
            hipos = np.nonzero(vals >= SBASE)[0].tolist()
            assert len(hipos) >= len(need), f"guard c={c} k={k}"
            rest = [j for j in range(kv) if j not in hipos[:len(need)]]
            newv = np.empty_like(vals)
            for t, j in zip(need, hipos):
                newv[t] = vals[j]
            rvals = [vals[j] for j in rest]
            oth = [t for t in range(kv) if t not in need]
            for t, vv in zip(oth, rvals):
                newv[t] = vv
            idx[c, slots] = newv

    # final verify: every call-final flat element is non-negative
    for c in range(C):
        for t in finals:
            assert idx[c, t * P + 127] >= SBASE, f"guard verify c={c} t={t}"
    v16 = idx - SBASE
    assert v16.min() >= -32768 and v16.max() < 32768
    pl.idx_lo = v16.astype(np.int16)
    pl.idx_hi = np.zeros((C, 8 * 16), np.int16)
    return pl


def build_plan_idma(src, dst, n_nodes, group_chunks=None):
    """Plan for the indirect_dma_start gather path.

    Single i32 index array per core (no int16 range split): nodes sorted by
    total in-degree, one slot grid, idx32[p, tile] = global table row of the
    source for slot (p, tile); pad slots point at the dummy row.
    """
    import os
    if group_chunks is None:
        group_chunks = int(os.environ.get("GAT_GROUP", "2"))
    pl = Plan()
    pl.idma = True
    npc = n_nodes // C
    assert npc * C == n_nodes
    chunks = -(-npc // P)
    npad = chunks * P
    slice_n = npad + 1              # + dummy row
    tbl_n = C * slice_n
    pl.npc, pl.chunks, pl.npad = npc, chunks, npad
    pl.slice_n, pl.tbl_n = slice_n, tbl_n

    owner = dst // npc
    perm = np.zeros((C, npad), np.int64)
    kcnt = np.zeros((C, npad), np.int32)
    edges = []                       # per core: (d_loc sorted, src)
    for c in range(C):
        m = owner == c
        d_loc = dst[m] - c * npc
        s = src[m]
        cnt = np.bincount(d_loc, minlength=npc)
        order = np.argsort(cnt, kind="stable")
        perm[c, :npc] = order
        perm[c, npc:] = npc
        kcnt[c, :npc] = cnt[order]
        o = np.argsort(d_loc, kind="stable")
        edges.append((d_loc[o], s[o]))

    kk = kcnt.reshape(C, chunks, P)
    tiles = np.maximum(kk.max(axis=(0, 2)), 1)   # [chunks]
    pl.tiles = tiles

    pl.group = group_chunks
    groups = [list(range(g, min(g + group_chunks, chunks)))
              for g in range(0, chunks, group_chunks)]
    groups.sort(key=lambda g: -int(tiles[g].sum()))
    if len(groups[-1]) > 1:
        last = groups.pop()
        groups.extend([[k] for k in last])
    pl.groups = groups

    invperm = np.zeros((C, npc), np.int64)
    for c in range(C):
        invperm[c, perm[c, :npc]] = np.arange(npad)[: npc]
    pos = (owner_all := np.arange(n_nodes) // npc) * slice_n \
        + invperm[owner_all, np.arange(n_nodes) % npc]
    pl.pos = pos
    pl.perm = perm

    dummy_global = npad              # core-0 slice's dummy row
    tile_base = np.concatenate([[0], np.cumsum(tiles)])
    ntile = int(tiles.sum())
    idx32 = np.full((C, P, ntile), dummy_global, np.int32)
    for c in range(C):
        d_loc, s = edges[c]
        if len(d_loc) == 0:
            continue
        posv = pos[s]
        slot = np.arange(len(d_loc)) - np.concatenate(
            [[0], np.cumsum(np.bincount(d_loc, minlength=npc))])[d_loc]
        ip = invperm[c, d_loc]
        ch, p = ip // P, ip % P
        idx32[c, p, tile_base[ch] + slot] = posv
    pl.idx32 = idx32
    pl.tile_base = tile_base
    pl.ntile = ntile
    return pl


# --------------------------------------------------------------------------
# device program
# --------------------------------------------------------------------------

def build_program(pl, in_dim, hid, heads, out_dim, num_devices=C):
    import os
    phase = os.environ.get("GAT_PHASE", "full")
    nd = heads * hid                 # 128 (layer-1 z width)
    assert nd == 128 and in_dim % P == 0
    kq = in_dim // P                 # k-chunks for layer-1 matmul
    chunks, npad, slice_n, tbl_n = pl.chunks, pl.npad, pl.slice_n, pl.tbl_n
    tlo, thi = pl.tlo, pl.thi
    ncols_lo = int(tlo.sum()) * 8    # idx sbuf cols
    ncols_hi = int(thi.sum()) * 8

    nocc = os.environ.get("GAT_NOCC", "0") == "1"
    maxidx = int(os.environ.get("GAT_MAXIDX", "1024"))
    scratch = int(os.environ.get("GAT_SCRATCH", "16384"))
    nc = bacc.Bacc("TRN2", target_bir_lowering=False, debug=False,
                   enable_asserts=False, num_devices=num_devices,
                   dynamic_dma_scratch_size=scratch)
    h_in = nc.dram_tensor("ht", [in_dim, npad], BF16, kind="ExternalInput")
    ilo_in = nc.dram_tensor("idx_lo", [P, max(ncols_lo, 8)], I16,
                            kind="ExternalInput")
    ihi_in = nc.dram_tensor("idx_hi", [P, max(ncols_hi, 8)], I16,
                            kind="ExternalInput")

    st1_in = nc.dram_tensor("stat1", [in_dim, 136], BF16,
                            kind="ExternalInput")
    st2_in = nc.dram_tensor("stat2", [nd, out_dim + 2], F32,
                            kind="ExternalInput")
    id_in = nc.dram_tensor("ident", [P, P], BF16, kind="ExternalInput")
    idf_in = nc.dram_tensor("identf", [P, P], F32, kind="ExternalInput")
    dum_in = nc.dram_tensor("dummyrow", [1, P], F32, kind="ExternalInput")
    out_dram = nc.dram_tensor("out", [npad, out_dim], F32,
                              kind="ExternalOutput")

    with tile.TileContext(nc) as tc:
        with (tc.tile_pool(name="const", bufs=1) as cpool,
              tc.tile_pool(name="dram", bufs=1, space="DRAM") as dpool,
              tc.tile_pool(name="work", bufs=3) as wpool,
              tc.tile_pool(name="gath", bufs=4) as gpool,
              tc.tile_pool(name="psA", bufs=1, space="PSUM") as pspool,
              tc.tile_pool(name="psE", bufs=2, space="PSUM") as pspoolE,
              tc.tile_pool(name="psZ", bufs=3, space="PSUM") as pspoolZ,
              tc.tile_pool(name="psB", bufs=1, space="PSUM") as pspool2):
            nc.gpsimd.load_library(mlp)

            # ---- constants / persistent tiles
            # one-shot const loads on the ACT hwdge queue: the sync queue
            # reaches Z1's first hT load immediately
            ident = cpool.tile([P, P], BF16)
            nc.scalar.dma_start(ident[:], id_in[:])
            identf = cpool.tile([P, P], F32)
            nc.scalar.dma_start(identf[:], idf_in[:])
            stat1 = cpool.tile([P, kq, 136], BF16)
            nc.scalar.dma_start(
                stat1[:], st1_in.ap().rearrange("(q p) n -> p q n", p=P))
            stat2 = cpool.tile([P, out_dim + 2], F32)
            nc.scalar.dma_start(stat2[:], st2_in[:])
            idx_lo = cpool.tile([P, max(ncols_lo, 8)], I16)
            nc.scalar.dma_start(idx_lo[:], ilo_in[:])
            idx_hi = cpool.tile([P, max(ncols_hi, 8)], I16)
            nc.scalar.dma_start(idx_hi[:], ihi_in[:])
            scores1 = cpool.tile([P, chunks, 8], F32)
            scores2 = cpool.tile([P, chunks], F32)

            slice1 = dpool.tile([slice_n, P], F32)
            table1 = dpool.tile([tbl_n, P], F32, addr_space="Shared")
            # layer-2 rows are 256B (z2 bf16 + es2/ed2 f32): half the
            # AllGather traffic of layer 1
            slice2 = dpool.tile([slice_n, 64], F32)
            table2 = dpool.tile([tbl_n, 64], F32, addr_space="Shared")

            # ---- phase Z1: own rows [z|es|ed] from host-transposed bf16 h
            # batch 4 chunks per DMA so the sync engine's issue+wait chain
            # (~2us per DMA) stops pacing the phase
            hT_view = h_in.ap().rearrange("(q p) n -> p q n", p=P)
            # AG packing/splitting: move only the meaningful 288B of each
            # 512B table1 row (72 f32: z bf16 + es/ed), via a strided
            # [C, rows, 72] out view; split the AG so the first AGH chunks
            # transfer while Z1 still computes the rest.
            # NOTE: Shared-DRAM tensors allow only ONE writer instruction, so
            # the AG cannot be split into overlapping halves (agh/agk >=
            # chunks disables the split); the byte-packing still applies.
            agpack = os.environ.get("GAT_AGPACK", "0") == "1"
            agh = int(os.environ.get("GAT_AGH", str(chunks)))
            t1v = table1[0:tbl_n, :].rearrange("(c r) w -> c r w", c=num_devices)

            def ag1(r0, r1):
                if agpack:
                    nc.gpsimd.collective_compute(
                        "AllGather", ALU.bypass,
                        replica_groups=[list(range(num_devices))],
                        ins=[slice1[r0:r1, 0:72].opt()],
                        outs=[t1v[:, r0:r1, 0:72].opt()])
                else:
                    nc.gpsimd.collective_compute(
                        "AllGather", ALU.bypass,
                        replica_groups=[list(range(num_devices))],
                        ins=[slice1[r0:r1, :].opt()],
                        outs=[t1v[:, r0:r1, :].opt()])

            ZB = 3
            ag1a_done = False
            for k0 in range(0, chunks, ZB):
                zb = min(ZB, chunks - k0)
                hT = wpool.tile([P, kq, zb * P], BF16, tag="hT")
                nc.sync.dma_start(
                    hT[:], hT_view[:, :, k0 * P:(k0 + zb) * P])
                rowt = wpool.tile([P, zb, P], F32, tag="rowt")
                psz = pspoolZ.tile([P, zb, 136], F32, tag="psz")
                for j in range(zb):
                    for q in range(kq):
                        nc.tensor.matmul(psz[:, j, :],
                                         hT[:, q, j * P:(j + 1) * P],
                                         stat1[:, q, :],
                                         start=(q == 0), stop=(q == kq - 1))
                nc.vector.memset(rowt[:, :, 72:P], 0.0)
                rbf = rowt.bitcast(BF16)
                nc.vector.tensor_copy(rbf[:, :, 0:P],
                                      psz[:, :, 0:P])
                nc.scalar.copy(rowt[:, :, 64:72], psz[:, :, 128:136])
                nc.vector.tensor_copy(scores1[:, k0:k0 + zb, :],
                                      psz[:, :, 128:136])
                # issue from ACT (which produced rowt's score cols) so the
                # sync queue only paces the hT loads
                nc.scalar.dma_start(
                    slice1[k0 * P:(k0 + zb) * P, :]
                    .rearrange("(c p) n -> p c n", p=P),
                    rowt[:])
                if not nocc and not ag1a_done and k0 + zb >= agh \
                        and k0 + zb < chunks:
                    ag1(0, (k0 + zb) * P)
                    ag1a_done = True
                    agh_rows = (k0 + zb) * P
            nc.sync.dma_start(slice1[npad:npad + 1, :], dum_in[:])
            if nocc:
                pass
            else:
                ag1(agh_rows if ag1a_done else 0, slice_n)

            elvl = int(os.environ.get("GAT_ELVL", "9"))

            # ---- edge phases
            def edge_phase(layer, table, scores_t, after_group=None):
                pay_w = P if layer == 1 else out_dim     # payload cols
                nh = 4 if layer == 1 else 1              # heads
                mw = pay_w + nh                          # payload + ex cols
                elem = 256 if layer == 1 else 128        # gathered bf16/row
                tbl_bf = table.bitcast(BF16)
                if getattr(pl, "signed_grid", False):
                    # signed idx: AP based at row SBASE, idx = pos - SBASE
                    lo_src = tbl_bf[SBASE:tbl_n, :]
                    hi_src = lo_src                      # thi == 0, unused
                else:
                    lo_src = tbl_bf[0:pl.hi_base, :]
                    hi_src = tbl_bf[pl.hi_base:tbl_n, :]
                for grp in pl.groups:
                    nlo = int(tlo[grp].sum())
                    nhi = int(thi[grp].sum())
                    gt = gpool.tile([P, nlo + nhi, elem], BF16, tag="gt")
                    mt = maxidx // P
                    for (src_ap, idxt, base_t, n_t, dst0) in (
                            (lo_src, idx_lo, int(pl.lo_tile_base[grp[0]]),
                             nlo, 0),
                            (hi_src, idx_hi, int(pl.hi_tile_base[grp[0]]),
                             nhi, nlo)):
                        done = 0
                        while done < n_t:
                            nt = min(n_t - done, mt)
                            c0 = (base_t + done) * 8
                            nc.gpsimd.dma_gather(
                                gt[:, dst0 + done:dst0 + done + nt, :],
                                src_ap, idxt[:, c0:c0 + nt * 8],
                                nt * P, nt * P, elem)
                            done += nt
                    gt32 = gt.bitcast(F32)
                    lo_b = int(pl.lo_tile_base[grp[0]])
                    hi_b = int(pl.hi_tile_base[grp[0]])
                    if elvl == 0:
                        sink = wpool.tile([P, 64], F32, tag="sink")
                        nc.vector.tensor_copy(sink[:], gt32[:, 0, 0:64])
                        nc.sync.dma_start(
                            slice2[grp[0] * P:(grp[0] + 1) * P, :], sink[:])
                        continue
                    for k in grp:
                        tl, th = int(tlo[k]), int(thi[k])
                        T = tl + th
                        ko_lo = int(pl.lo_tile_base[k]) - lo_b
                        ko_hi = nlo + int(pl.hi_tile_base[k]) - hi_b
                        # e = es[src] + ed[dst]
                        e32 = wpool.tile([P, T, nh], F32, tag="e32")
                        for (off, cnt, eo) in ((ko_lo, tl, 0), (ko_hi, th, tl)):
                            if cnt == 0:
                                continue
                            if layer == 1:
                                esv = gt32[:, off:off + cnt, 64:68]
                                edv = (scores_t[:, k, 4:8].unsqueeze(1)
                                       .broadcast_to([P, cnt, 4]))
                            else:
                                esv = gt32[:, off:off + cnt, 32:33]
                                edv = (scores_t[:, k:k + 1].unsqueeze(1)
                                       .broadcast_to([P, cnt, 1]))
                            nc.vector.tensor_tensor(
                                e32[:, eo:eo + cnt, :], esv, edv, ALU.add)
                        ef = e32[:].rearrange("p t h -> p (t h)")
                        lr = wpool.tile([P, T, nh], F32, tag="lr")
                        lrf = lr[:].rearrange("p t h -> p (t h)")
                        nc.vector.scalar_tensor_tensor(
                            lrf, ef, 0.01, ef, ALU.mult, ALU.max)
                        # payex: [pay | ex] so one matmul accumulates the
                        # weighted sum and the softmax denominator together
                        payex = wpool.tile([P, T, mw], BF16, tag="payex")
                        nc.scalar.activation(
                            payex[:, 0:T, pay_w:mw], lr[:], AF.Exp)
                        if elvl == 1:
                            sink = wpool.tile([P, 64], F32, tag="sink")
                            nc.vector.memset(sink[:], 0.0)
                            nc.sync.dma_start(
                                slice2[k * P:(k + 1) * P, :], sink[:])
                            continue
                        psz = pspoolE.tile([P, mw], F32, tag="psE")
                        # two passes: all DVE multiplies first, then all
                        # matmuls — keeps DVE of chunk k+1 overlapping the
                        # TensorE accumulation of chunk k
                        for t in range(T):
                            col = (ko_lo + t) if t < tl else (ko_hi + t - tl)
                            if layer == 1:
                                zin = gt[:, col, 0:P].rearrange(
                                    "p (a b) -> p a b", a=4)
                                exv = (payex[:, t, pay_w:mw].unsqueeze(2)
                                       .broadcast_to([P, 4, 32]))
                                nc.vector.tensor_tensor(
                                    payex[:, t, 0:pay_w].rearrange(
                                        "p (a b) -> p a b", a=4),
                                    zin, exv, ALU.mult)
                            else:
                                zin = gt[:, col, 0:out_dim]
                                exv = (payex[:, t, pay_w:mw]
                                       .broadcast_to([P, out_dim]))
                                nc.vector.tensor_tensor(
                                    payex[:, t, 0:pay_w], zin, exv, ALU.mult)
                        for t in range(T):
                            nc.tensor.matmul(psz[:], ident[:], payex[:, t, :],
                                             start=(t == 0), stop=(t == T - 1))
                        # epilogue
                        den = wpool.tile([P, nh], F32, tag="den")
                        nc.vector.tensor_scalar_add(den[:], psz[:, pay_w:mw],
                                                    1e-30)
                        rec = wpool.tile([P, nh], F32, tag="rec")
                        nc.vector.reciprocal(rec[:], den[:])
                        if layer == 1:
                            h1 = wpool.tile([P, P], F32, tag="h1")
                            rv = (rec[:].unsqueeze(2)
                                  .broadcast_to([P, 4, 32]))
                            nc.vector.tensor_tensor(
                                h1[:].rearrange("p (a b) -> p a b", a=4),
                                psz[:, 0:P].rearrange("p (a b) -> p a b", a=4),
                                rv, ALU.mult)
                            # elu(x) = max(x, exp(min(x,0)) - 1)
                            mn = wpool.tile([P, P], F32, tag="mn")
                            nc.vector.tensor_scalar_min(mn[:], h1[:], 0.0)
                            em = wpool.tile([P, P], F32, tag="em")
                            nc.scalar.activation(em[:], mn[:], AF.Exp)
                            h1e = wpool.tile([P, P], F32, tag="h1e")
                            nc.vector.scalar_tensor_tensor(
                                h1e[:], em[:], -1.0, h1[:], ALU.add, ALU.max)
                            # z2 = h1e @ [W2 | W2 a2s | W2 a2d]
                            pst2 = pspool.tile([P, P], F32, tag="pst2")
                            nc.tensor.matmul(pst2[:], h1e[:], identf[:],
                                             is_transpose=True)
                            h1T = wpool.tile([P, P], F32, tag="h1T")
                            nc.vector.tensor_copy(h1T[:], pst2[:])
                            psz2 = pspool2.tile([P, out_dim + 2], F32,
                                                tag="psz2")
                            nc.tensor.matmul(psz2[:], h1T[:], stat2[:])
                            # layer-2 row: z2 as bf16 in bytes [0,128),
                            # es2/ed2 f32 at f32 cols 32:34 (256B rows)
                            row2 = wpool.tile([P, 64], F32, tag="row2")
                            r2bf = row2.bitcast(BF16)
                            nc.vector.tensor_copy(r2bf[:, 0:out_dim],
                                                  psz2[:, 0:out_dim])
                            nc.scalar.copy(row2[:, 32:34],
                                           psz2[:, out_dim:out_dim + 2])
                            nc.vector.tensor_copy(scores2[:, k:k + 1],
                                                  psz2[:, out_dim + 1:
                                                       out_dim + 2])
                            nc.sync.dma_start(slice2[k * P:(k + 1) * P, 0:34],
                                              row2[:, 0:34])
                        else:
                            orow = wpool.tile([P, out_dim], F32, tag="orow")
                            rv = rec[:].broadcast_to([P, out_dim])
                            nc.vector.tensor_tensor(orow[:], psz[:, 0:out_dim],
                                                    rv, ALU.mult)
                            nc.sync.dma_start(
                                out_dram[k * P:(k + 1) * P, :], orow[:])
                    if after_group is not None:
                        after_group(grp)

            if phase == "z1":
                nc.sync.dma_start(out_dram[0:npad, :],
                                  slice1[0:npad, 0:out_dim])
            elif phase == "e1":
                edge_phase(1, table1, scores1)
                nc.sync.dma_start(out_dram[0:npad, :],
                                  slice2[0:npad, 0:out_dim])
            else:
                # constant dummy row: write before the edge phase so AG2
                # never waits on a late tiny DMA
                nc.sync.dma_start(slice2[npad:npad + 1, :], dum_in[:, 0:64])
                t2v = table2[0:tbl_n, :].rearrange("(c r) w -> c r w",
                                                   c=num_devices)

                def ag2(r0, r1):
                    if agpack:
                        nc.gpsimd.collective_compute(
                            "AllGather", ALU.bypass,
                            replica_groups=[list(range(num_devices))],
                            ins=[slice2[r0:r1, 0:34].opt()],
                            outs=[t2v[:, r0:r1, 0:34].opt()])
                    else:
                        nc.gpsimd.collective_compute(
                            "AllGather", ALU.bypass,
                            replica_groups=[list(range(num_devices))],
                            ins=[slice2[r0:r1, :].opt()],
                            outs=[t2v[:, r0:r1, :].opt()])

                # split AG2: chunks >= agk finish first (groups run
                # largest-tilesum-first = high-degree = high chunk index),
                # so their rows AllGather while low chunks still compute
                agk = int(os.environ.get("GAT_AGK", str(chunks)))
                pend_hi = set(k for k in range(chunks) if k >= agk)
                st = {"emitted": False}

                def on_group(grp):
                    if nocc or st["emitted"]:
                        return
                    pend_hi.difference_update(grp)
                    if not pend_hi:
                        ag2(agk * P, slice_n)
                        st["emitted"] = True

                edge_phase(1, table1, scores1,
                           after_group=None if agk >= chunks else on_group)
                if nocc:
                    pass
                else:
                    if st["emitted"]:
                        ag2(0, agk * P)
                    else:
                        ag2(0, slice_n)
                edge_phase(2, table2, scores2)

    nc.compile()
    return nc


# --------------------------------------------------------------------------
# host wrapper
# --------------------------------------------------------------------------

def make_inputs(pl, h, W1, a1, W2, a2, in_dim, hid, heads, out_dim):
    n_nodes = h.shape[0]
    npc, npad = pl.npc, pl.npad
    nd = heads * hid
    # stationaries
    st1 = np.zeros((in_dim, 136), np.float32)
    for hh in range(heads):
        st1[:, hh * hid:(hh + 1) * hid] = W1[hh]
        st1[:, 128 + hh] = W1[hh] @ a1[hh, :hid]
        st1[:, 132 + hh] = W1[hh] @ a1[hh, hid:]
    st2 = np.zeros((nd, out_dim + 2), np.float32)
    st2[:, :out_dim] = W2[0]
    st2[:, out_dim] = W2[0] @ a2[0, :out_dim]
    st2[:, out_dim + 1] = W2[0] @ a2[0, out_dim:]
    dummy = np.zeros((1, 128), np.float32)
    dummy[0, 64:72] = -1e30
    dummy[0, 32] = -1e30   # layer-2 es slot (256B rows)
    identb = np.eye(128, dtype=BF)
    identf = np.eye(128, dtype=np.float32)

    ncols_lo = int(pl.tlo.sum()) * 8
    ncols_hi = int(pl.thi.sum()) * 8
    in_maps = []
    st1b = st1.astype(BF)
    for c in range(C):
        hp = np.zeros((npad, in_dim), np.float32)
        valid = pl.perm[c] < npc
        hp[valid] = h[c * npc + pl.perm[c][valid]]
        ht = np.ascontiguousarray(hp.T).astype(BF)
        ilo = wrap_idx(pl.idx_lo[c]) if ncols_lo else \
            np.zeros((128, 8), np.int16)
        ihi = wrap_idx(pl.idx_hi[c]) if ncols_hi else \
            np.zeros((128, 8), np.int16)
        in_maps.append({
            "ht": ht, "idx_lo": ilo, "idx_hi": ihi,
            "stat1": st1b, "stat2": st2, "ident": identb, "identf": identf,
            "dummyrow": dummy,
        })
    return in_maps


def unpermute(pl, outs, n_nodes, out_dim):
    npc = pl.npc
    full = np.zeros((n_nodes, out_dim), np.float32)
    for c in range(C):
        valid = pl.perm[c] < npc
        full[c * npc + pl.perm[c][valid]] = outs[c][valid]
    return full


def gat_run(h, src, dst, W1, a1, W2, a2, runner):
    """Full pipeline; `runner(nc, in_maps)` -> list of per-core {'out': arr}."""
    import os
    n_nodes, in_dim = h.shape
    heads, _, hid = W1.shape
    out_dim = W2.shape[2]
    if os.environ.get("GAT_SG", "1") == "1":
        pl = build_plan_sg(np.asarray(src), np.asarray(dst), n_nodes)
    else:
        pl = build_plan(np.asarray(src), np.asarray(dst), n_nodes)
    nc = build_program(pl, in_dim, hid, heads, out_dim)
    in_maps = make_inputs(pl, np.asarray(h, np.float32), np.asarray(W1),
                          np.asarray(a1), np.asarray(W2), np.asarray(a2),
                          in_dim, hid, heads, out_dim)
    outs = runner(nc, in_maps)
    return unpermute(pl, [o["out"] for o in outs], n_nodes, out_dim)


def hw_runner(nc, in_maps, trace=None):
    import os
    from concourse.bass_utils import run_bass_kernel_spmd
    if trace is None:
        trace = os.environ.get("GAT_TRACE", "0") == "1"
    res = run_bass_kernel_spmd(nc, in_maps, core_ids=list(range(C)),
                               trace=trace)
    hw_runner.last = res
    return res.results


def kernel(**inputs):
    out = gat_run(inputs["h"], inputs["src"], inputs["dst"], inputs["W1"],
                  inputs["a1"], inputs["W2"], inputs["a2"], hw_runner)
    return out



# revision 35
# speedup vs baseline: 1.0050x; 1.0050x over previous
"""2-layer GAT on 8 Trainium2 NeuronCores (Bass/Tile).

Strategy (dst-partitioned, gather-based):
- Nodes are partitioned contiguously across 8 cores by destination; each core
  handles all edges whose dst lands in its range, so per-core outputs and the
  per-destination softmax segments are fully local (no cross-core reduction).
- Per layer, each core computes node rows [z | es | ed] for its own nodes with
  TensorE matmuls, the 8 slices are AllGather-ed into a replicated DRAM table,
  and each core uses `dma_gather` (512B rows) to fetch z/es of every edge's
  source node.
- Edges are laid out host-side in a (node-partition x slot) grid: each 128-node
  chunk gets T slot-tiles; tile t holds the t-th incoming edge of each node in
  partition p. Nodes are bucketed by in-degree (split into low/high source
  ranges for int16 gather indices, superblock-sorted on both counts) so
  padding is small. Pad slots point at a dummy table row with es = -1e30,
  which exp() maps to an exact 0 weight.
- Per slot-tile: ex = exp(leaky_relu(es_src + ed_dst)) on DVE/ACT written into
  a fused [pay | ex] bf16 tile, payload ex*z on DVE, then ONE
  identity-stationary matmul per tile accumulates the weighted sum and the
  softmax denominator together into PSUM. A per-chunk epilogue divides,
  applies elu (layer 1), and computes the next layer's node rows.
- Layer-1 table rows are 512B (z bf16 + es/ed f32); layer-2 rows are 256B
  (z2 bf16 + es2/ed2 f32), halving the second AllGather. Tables are Shared
  DRAM (fast collective path). Groups are processed largest-first so the
  drain into each AllGather is short. dma_gather is q7 desc-gen bound
  (~7.8 ns/row); prep/trigger splitting and >1024-idx calls do NOT help.

Landed 2026-08-10 (2.31ms -> 2.00ms, rel_err unchanged 0.00629):
- SIGNED-IDX SINGLE GRID (build_plan_sg, GAT_SG=1 default): the q7 gather
  address math is IVP_MULUSAN (unsigned stride x SIGNED idx), HW-verified
  with bench_neg.py - so idx int16 = pos - 32768 with the gather AP based
  at table row 32768 reaches all 50184 rows. Kills the lo/hi range split:
  one degree sort -> 813 tiles vs 924 (-12% gather chain, -314us).
  Constraint: the q7 TRIMS a trailing run of NEGATIVE idx per call, so the
  last flat slot (call-final tile, p127) of every gather call must hold a
  value >= 32768 or the dummy; enforced by reordering the p127 node's own
  edges among its slots (order-invariant math) with node swaps as
  fallback; build asserts verify every call-final element.
- AG PACKING (GAT_AGPACK, default OFF): moving only the meaningful row
  bytes via strided collective APs (table1: 72 of 128 f32/row) passes the
  Tile sim but CRASHES the axon/PJRT runtime (JaxRuntimeError INTERNAL at
  execute) - the CC lowering does not accept 3-dim strided views. Keep 0.
- Shared-DRAM tensors allow only ONE writer instruction: a table cannot
  be filled by two split AllGathers (tried for compute/AG overlap; the
  Tile event loop rejects it). Split-AG would need per-half tables plus a
  gather source spanning both - not expressible as one AP.

Perf findings (trace-verified, 2026-08-10 session; HW 2.31ms):
- DMAGatherAnt on GpSimd = 2.04ms busy (88% of span), 276 calls x ~8.6us
  per 1024 idx = 8.4 ns/idx, uniform (no per-call fixed cost: a 537-idx
  call took 4.6us). DMA engines only ~45% busy (~53ns/desc = 3.3ns/row
  wall over 16 engines). Everything else (PE 530us, DVE 833us, 2 AGs
  160us) overlaps under the gather serial chain.
- dma_gather executes on ONE q7 core pair picked by queue_num
  (dma_gather.cpp: cpu_id/2 == queue_num); the Pool engine serializes
  instructions, so multi-queue calls do NOT overlap desc-gen. A custom
  4-queue q7 kernel would fix it, but the container has NO xtensa
  toolchain (library_overlay.build_library needs nix ucodeEnvWithTools;
  prebuilt-deps tarballs are LFS stubs) - dead end here.
- The cost model's 0.34ns/desc is the CounterMachine dma_start path,
  not dma_gather's software desc loop (scalar idx unpack + per-16-lane
  pushes) - don't trust it for gathers.
- indirect_dma_start (InstDMACopy + DynamicAP, bench_gather.py):
  desc-gen is 0.56ns/row (INDIRECT1D 2.3us per 4096 rows - 15x better
  than dma_gather!) BUT (a) with a contiguous [P,T,128] f32 dest the
  lowering MERGES rows into 1024B descs consuming one index per TWO rows
  (wrong data); (b) with non-contiguous rows ([P,T,132] view 0:128) the
  gathered rows didn't match the index set at all (mapping unresolved);
  (c) DMA transfer is ~330ns/desc = 20.7ns/row over 16 engines (engines
  99% busy, un-concatenated packets) vs dma_gather's 3.3ns/row
  single_packet stream. A ~20% tile offload hybrid (isplits/idx32_* in
  the plan are groundwork) would only pay if (b) gets solved - derive
  the HW index-consumption order first with bench_gather.py's probe.
- GAT_SB=12 is the tile-count optimum (924); global degree-balanced
  round-robin node->core assignment makes padding WORSE (1407 tiles).
- ap_gather/gather_transpose (free-dim SBUF gather) are the wrong layout
  (edges land on free dim; PE transposes to fix cost ~3x the PE budget),
  and likely ~100cyc/4idx on cayman (ReadOverlap=0 pitfall).
"""
import sys

sys.path.insert(0, "/opt/trn_rl_repo")

import numpy as np
import ml_dtypes

import concourse.bass as bass
import concourse.bacc as bacc
import concourse.mybir as mybir
import concourse.tile as tile
from concourse.library_config import mlp

F32 = mybir.dt.float32
BF16 = mybir.dt.bfloat16
I16 = mybir.dt.int16
AF = mybir.ActivationFunctionType
ALU = mybir.AluOpType
BF = ml_dtypes.bfloat16

C = 8          # cores
P = 128        # partitions


# --------------------------------------------------------------------------
# host-side preprocessing
# --------------------------------------------------------------------------

class Plan:
    """Host-computed layout shared by the program builder and per-core data."""


def build_plan(src, dst, n_nodes, group_chunks=None):
    import os
    if group_chunks is None:
        group_chunks = int(os.environ.get("GAT_GROUP", "2"))
    pl = Plan()
    npc = n_nodes // C
    assert npc * C == n_nodes
    chunks = -(-npc // P)
    npad = chunks * P
    slice_n = npad + 1              # + dummy row
    tbl_n = C * slice_n
    # low/high split for int16 gather indices
    hi_core = (C + 1) // 2          # cores [0,hi_core) low, rest high
    while hi_core * slice_n > 32768:
        hi_core -= 1
    assert (C - hi_core) * slice_n <= 32768, "table too large for 2-way split"
    hi_base = hi_core * slice_n
    pl.npc, pl.chunks, pl.npad = npc, chunks, npad
    pl.slice_n, pl.tbl_n, pl.hi_core, pl.hi_base = slice_n, tbl_n, hi_core, hi_base

    owner = dst // npc
    src_owner = src // npc
    is_lo = src_owner < hi_core

    # per-core, per-node in-edge lists split by src range
    perm = np.zeros((C, npad), np.int64)        # processing order -> local id
    klo = np.zeros((C, npad), np.int32)
    khi = np.zeros((C, npad), np.int32)
    edges_lo = []                                # per core: [n_lo_edges] srcs sorted by (dstlocal)
    edges_hi = []
    sb = int(os.environ.get("GAT_SB", "12")) * P  # superblock resort size
    for c in range(C):
        m = owner == c
        d_loc = dst[m] - c * npc
        s = src[m]
        lo_m = is_lo[m]
        cnt_lo = np.bincount(d_loc[lo_m], minlength=npc)
        cnt_hi = np.bincount(d_loc[~lo_m], minlength=npc)
        order = np.lexsort((cnt_hi, cnt_lo))     # sort nodes by (klo, khi)
        if sb > 0:
            # re-sort by khi within superblocks: keeps klo nearly sorted
            # (narrow range per block) while making khi sorted within each
            # block, shrinking both per-chunk maxima.
            kh_o = cnt_hi[order]
            for b in range(0, npc, sb):
                e = min(b + sb, npc)
                sub = np.argsort(kh_o[b:e], kind="stable")
                order[b:e] = order[b:e][sub]
        perm[c, :npc] = order
        perm[c, npc:] = npc                      # phantom marker
        klo[c, :npc] = cnt_lo[order]
        khi[c, :npc] = cnt_hi[order]
        # edge lists grouped by local dst: sort edges by d_loc
        o_lo = np.argsort(d_loc[lo_m], kind="stable")
        o_hi = np.argsort(d_loc[~lo_m], kind="stable")
        edges_lo.append((d_loc[lo_m][o_lo], s[lo_m][o_lo]))
        edges_hi.append((d_loc[~lo_m][o_hi], s[~lo_m][o_hi]))

    # global per-chunk tile counts
    kl = klo.reshape(C, chunks, P)
    kh = khi.reshape(C, chunks, P)
    tlo = np.maximum(kl.max(axis=(0, 2)), 1)     # [chunks], >= 1
    thi = kh.max(axis=(0, 2))                    # [chunks]
    pl.tlo, pl.thi = tlo, thi

    # groups of chunks per gather call, processed largest-first so the
    # epilogue drain before each AllGather is a small chunk's chain
    pl.group = group_chunks
    groups = [list(range(g, min(g + group_chunks, chunks)))
              for g in range(0, chunks, group_chunks)]
    tilesum = tlo + thi
    groups.sort(key=lambda g: -int(tilesum[g].sum()))
    # split the smallest (last-processed) group into singletons: the
    # epilogue drain into each AllGather and the final tail are one small
    # chunk's chain instead of two
    if len(groups[-1]) > 1:
        last = groups.pop()
        groups.extend([[k] for k in last])
    pl.groups = groups

    # per-group tile counts routed to the indirect-DMA path (tail tiles of
    # the group's lo/hi ranges): Pool desc-gen is ~15x cheaper there, DMA
    # per-row ~6x dearer, so a small fraction balances the two.
    ifrac = float(os.environ.get("GAT_IFRAC", "0.2"))
    pl.isplits = [(int(ifrac * int(tlo[g].sum())),
                   int(ifrac * int(thi[g].sum()))) for g in groups]

    # position of original node v in the table: owner*slice_n + invperm
    invperm = np.zeros((C, npc), np.int64)
    for c in range(C):
        invperm[c, perm[c, :npc]] = np.arange(npad)[: npc]
    pos = (owner_all := np.arange(n_nodes) // npc) * slice_n \
        + invperm[owner_all, np.arange(n_nodes) % npc]
    pl.pos = pos
    pl.perm = perm

    # build per-core int16 gather index arrays (tile-major inside groups)
    dummy_rel = npad                            # dummy row, relative to base
    idx_lo = np.full((C, int(tlo.sum()) * P), dummy_rel, np.int32)
    idx_hi = np.full((C, int(thi.sum()) * P), dummy_rel, np.int32)
    lo_tile_base = np.concatenate([[0], np.cumsum(tlo)])   # per chunk
    hi_tile_base = np.concatenate([[0], np.cumsum(thi)])
    for c in range(C):
        for (d_loc, s), karr, idx, tbase, tcnt, base_off in (
            (edges_lo[c], kl[c], idx_lo[c], lo_tile_base, tlo, 0),
            (edges_hi[c], kh[c], idx_hi[c], hi_tile_base, thi, pl.hi_base),
        ):
            if len(d_loc) == 0:
                continue
            posv = pos[s] - base_off
            # slot index of each edge within its node's list (0..k-1)
            # edges are sorted by d_loc; slot = running index within node
            slot = np.arange(len(d_loc)) - np.concatenate(
                [[0], np.cumsum(np.bincount(d_loc, minlength=npc))])[d_loc]
            # node -> (chunk, partition) via invperm
            ip = invperm[c, d_loc]
            ch, p = ip // P, ip % P
            flat = (tbase[ch] + slot) * P + p
            idx[flat] = posv
    assert idx_lo.max() < 32768 and idx_hi.max() < 32768
    pl.idx_lo, pl.idx_hi = idx_lo.astype(np.int16), idx_hi.astype(np.int16)
    pl.lo_tile_base, pl.hi_tile_base = lo_tile_base, hi_tile_base

    # absolute-row i32 index grids [P, tiles] for the indirect-DMA hybrid:
    # idx32_*[p, tile] = global table row for slot (p, tile); pads -> dummy
    # (row npad inside core-0's slice).
    ntl, nth = int(tlo.sum()), int(thi.sum())
    idx32_lo = np.full((C, P, max(ntl, 1)), npad, np.int32)
    idx32_hi = np.full((C, P, max(nth, 1)), npad, np.int32)
    for c in range(C):
        f = idx_lo[c].astype(np.int64)
        m = f != dummy_rel
        flat = np.nonzero(m)[0]
        idx32_lo[c, flat % P, flat // P] = f[m]
        f = idx_hi[c].astype(np.int64)
        m = f != dummy_rel
        flat = np.nonzero(m)[0]
        idx32_hi[c, flat % P, flat // P] = f[m] + hi_base
    pl.idx32_lo, pl.idx32_hi = idx32_lo, idx32_hi
    return pl


def wrap_idx(arr):
    """[n] int16 -> [128, n/16] wrapped + replicated across the 8 q7 cores."""
    n = arr.shape[0]
    assert n % 16 == 0
    w = arr.reshape(n // 16, 16).T               # [16, n/16]
    return np.tile(w, (8, 1)).copy()


SBASE = 32768   # gather AP base row: idx int16 = pos - SBASE (signed)


def build_plan_sg(src, dst, n_nodes, group_chunks=None):
    """Single-grid plan: signed int16 gather indices (HW-verified: the q7
    address mul-acc is IVP_MULUSAN = unsigned*SIGNED, so idx = pos - 32768
    with the gather AP based at table row 32768 spans all 50184 rows).
    Kills the lo/hi split: one degree sort -> 813 tiles vs 924.

    Trailing-trim guard: the q7 drops a TRAILING run of negative idx per
    call, so the last flat slot (tile call_end, p127) of every gather call
    must hold a value >= SBASE (high row or the dummy). Enforced by
    reordering the p127 node's own edges (order-invariant math) and, where
    that can't work, swapping which node sits at p127 of the chunk.
    """
    import os
    if group_chunks is None:
        group_chunks = int(os.environ.get("GAT_GROUP", "2"))
    mt = int(os.environ.get("GAT_MAXIDX", "1024")) // P
    pl = Plan()
    npc = n_nodes // C
    assert npc * C == n_nodes
    chunks = -(-npc // P)
    npad = chunks * P
    slice_n = npad + 1
    tbl_n = C * slice_n
    pl.npc, pl.chunks, pl.npad = npc, chunks, npad
    pl.slice_n, pl.tbl_n = slice_n, tbl_n
    pl.signed_grid = True
    pl.hi_core, pl.hi_base = C, C * slice_n      # unused; keep fields valid

    owner = dst // npc
    deg = np.zeros((C, npc), np.int64)
    d_locs, srcs = [], []
    for c in range(C):
        m = owner == c
        d_loc = dst[m] - c * npc
        s = src[m]
        o = np.argsort(d_loc, kind="stable")
        d_locs.append(d_loc[o])
        srcs.append(s[o])
        deg[c] = np.bincount(d_loc, minlength=npc)

    perm = np.zeros((C, npad), np.int64)
    for c in range(C):
        perm[c, :npc] = np.argsort(deg[c], kind="stable")
        perm[c, npc:] = npc

    kcnt = np.zeros((C, npad), np.int32)
    for c in range(C):
        kcnt[c, :npc] = deg[c][perm[c, :npc]]
    kk = kcnt.reshape(C, chunks, P)
    tiles = np.maximum(kk.max(axis=(0, 2)), 1)
    tile_base = np.concatenate([[0], np.cumsum(tiles)])
    pl.tlo, pl.thi = tiles.astype(np.int64), np.zeros(chunks, np.int64)
    pl.lo_tile_base = tile_base
    pl.hi_tile_base = np.zeros(chunks + 1, np.int64)

    # groups (largest first; last multi-chunk group split to singletons)
    pl.group = group_chunks
    groups = [list(range(g, min(g + group_chunks, chunks)))
              for g in range(0, chunks, group_chunks)]
    groups.sort(key=lambda g: -int(tiles[g].sum()))
    if len(groups[-1]) > 1:
        last = groups.pop()
        groups.extend([[k] for k in last])
    pl.groups = groups
    pl.isplits = [(0, 0)] * len(groups)

    # call-final global tiles (per group range, mt tiles per gather call)
    finals = set()
    for g in groups:
        b, n = int(tile_base[g[0]]), int(tiles[g].sum())
        done = 0
        while done < n:
            nt = min(n - done, mt)
            finals.add(b + done + nt - 1)
            done += nt

    def make_pos():
        invperm = np.zeros((C, npc), np.int64)
        for c in range(C):
            invperm[c, perm[c, :npc]] = np.arange(npc)
        ow = np.arange(n_nodes) // npc
        return invperm, ow * slice_n + invperm[ow, np.arange(n_nodes) % npc]

    invperm, pos = make_pos()

    # guard feasibility: for each (core, chunk) the p127 node (rank
    # k*128+127) must own >= (#guards below its degree) edges with
    # pos[src] >= SBASE. Swap in a feasible node where needed.
    def nhigh_of(c):
        hi = (pos[srcs[c]] >= SBASE).astype(np.int64)
        return np.bincount(d_locs[c], weights=hi, minlength=npc).astype(
            np.int64)

    for it in range(3):
        swapped = 0
        for c in range(C):
            nh = nhigh_of(c)
            for k in range(chunks):
                gset = [t - int(tile_base[k]) for t in finals
                        if tile_base[k] <= t < tile_base[k] + tiles[k]]
                if not gset:
                    continue
                r = k * P + 127
                if r >= npc:
                    continue

                def ok(u):
                    return nh[u] >= sum(1 for t in gset if t < deg[c][u])

                v = perm[c, r]
                if ok(v):
                    continue
                band = perm[c, k * P:(k + 1) * P]
                cand = [j for j in range(P - 1) if ok(band[j])]
                assert cand, f"no guard-feasible node c={c} k={k}"
                j = cand[-1]
                perm[c, k * P + j], perm[c, r] = perm[c, r], perm[c, k * P + j]
                swapped += 1
        if swapped == 0:
            break
        invperm, pos = make_pos()   # swaps moved table rows; recompute
    pl.pos, pl.perm = pos, perm

    # build flat idx (absolute rows), then guard-reorder p127 nodes' slots
    dummy_global = 6 * slice_n + npad            # high slice -> idx >= 0
    assert dummy_global >= SBASE
    ntile = int(tiles.sum())
    idx = np.full((C, ntile * P), dummy_global, np.int64)
    for c in range(C):
        d_loc, s = d_locs[c], srcs[c]
        posv = pos[s]
        slot = np.arange(len(d_loc)) - np.concatenate(
            [[0], np.cumsum(np.bincount(d_loc, minlength=npc))])[d_loc]
        ip = invperm[c, d_loc]
        ch, p = ip // P, ip % P
        idx[c, (tile_base[ch] + slot) * P + p] = posv
        for k in range(chunks):
            gset = [t - int(tile_base[k]) for t in finals
                    if tile_base[k] <= t < tile_base[k] + tiles[k]]
            r = k * P + 127
            if not gset or r >= npc:
                continue
            v = perm[c, r]
            kv = int(deg[c][v])
            slots = (tile_base[k] + np.arange(kv)) * P + 127
            vals = idx[c, slots]
            need = [t for t in gset if t < kv]
            hipos = np.nonzero(vals >= SBASE)[0].tolist()
            assert len(hipos) >= len(need), f"guard c={c} k={k}"
            rest = [j for j in range(kv) if j not in hipos[:len(need)]]
            newv = np.empty_like(vals)
            for t, j in zip(need, hipos):
                newv[t] = vals[j]
            rvals = [vals[j] for j in rest]
            oth = [t for t in range(kv) if t not in need]
            for t, vv in zip(oth, rvals):
                newv[t] = vv
            idx[c, slots] = newv

    # final verify: every call-final flat element is non-negative
    for c in range(C):
        for t in finals:
            assert idx[c, t * P + 127] >= SBASE, f"guard verify c={c} t={t}"
    v16 = idx - SBASE
    assert v16.min() >= -32768 and v16.max() < 32768
    pl.idx_lo = v16.astype(np.int16)
    pl.idx_hi = np.zeros((C, 8 * 16), np.int16)
    return pl


def build_plan_idma(src, dst, n_nodes, group_chunks=None):
    """Plan for the indirect_dma_start gather path.

    Single i32 index array per core (no int16 range split): nodes sorted by
    total in-degree, one slot grid, idx32[p, tile] = global table row of the
    source for slot (p, tile); pad slots point at the dummy row.
    """
    import os
    if group_chunks is None:
        group_chunks = int(os.environ.get("GAT_GROUP", "2"))
    pl = Plan()
    pl.idma = True
    npc = n_nodes // C
    assert npc * C == n_nodes
    chunks = -(-npc // P)
    npad = chunks * P
    slice_n = npad + 1              # + dummy row
    tbl_n = C * slice_n
    pl.npc, pl.chunks, pl.npad = npc, chunks, npad
    pl.slice_n, pl.tbl_n = slice_n, tbl_n

    owner = dst // npc
    perm = np.zeros((C, npad), np.int64)
    kcnt = np.zeros((C, npad), np.int32)
    edges = []                       # per core: (d_loc sorted, src)
    for c in range(C):
        m = owner == c
        d_loc = dst[m] - c * npc
        s = src[m]
        cnt = np.bincount(d_loc, minlength=npc)
        order = np.argsort(cnt, kind="stable")
        perm[c, :npc] = order
        perm[c, npc:] = npc
        kcnt[c, :npc] = cnt[order]
        o = np.argsort(d_loc, kind="stable")
        edges.append((d_loc[o], s[o]))

    kk = kcnt.reshape(C, chunks, P)
    tiles = np.maximum(kk.max(axis=(0, 2)), 1)   # [chunks]
    pl.tiles = tiles

    pl.group = group_chunks
    groups = [list(range(g, min(g + group_chunks, chunks)))
              for g in range(0, chunks, group_chunks)]
    groups.sort(key=lambda g: -int(tiles[g].sum()))
    if len(groups[-1]) > 1:
        last = groups.pop()
        groups.extend([[k] for k in last])
    pl.groups = groups

    invperm = np.zeros((C, npc), np.int64)
    for c in range(C):
        invperm[c, perm[c, :npc]] = np.arange(npad)[: npc]
    pos = (owner_all := np.arange(n_nodes) // npc) * slice_n \
        + invperm[owner_all, np.arange(n_nodes) % npc]
    pl.pos = pos
    pl.perm = perm

    dummy_global = npad              # core-0 slice's dummy row
    tile_base = np.concatenate([[0], np.cumsum(tiles)])
    ntile = int(tiles.sum())
    idx32 = np.full((C, P, ntile), dummy_global, np.int32)
    for c in range(C):
        d_loc, s = edges[c]
        if len(d_loc) == 0:
            continue
        posv = pos[s]
        slot = np.arange(len(d_loc)) - np.concatenate(
            [[0], np.cumsum(np.bincount(d_loc, minlength=npc))])[d_loc]
        ip = invperm[c, d_loc]
        ch, p = ip // P, ip % P
        idx32[c, p, tile_base[ch] + slot] = posv
    pl.idx32 = idx32
    pl.tile_base = tile_base
    pl.ntile = ntile
    return pl


# --------------------------------------------------------------------------
# device program
# --------------------------------------------------------------------------

def build_program(pl, in_dim, hid, heads, out_dim, num_devices=C):
    import os
    phase = os.environ.get("GAT_PHASE", "full")
    nd = heads * hid                 # 128 (layer-1 z width)
    assert nd == 128 and in_dim % P == 0
    kq = in_dim // P                 # k-chunks for layer-1 matmul
    chunks, npad, slice_n, tbl_n = pl.chunks, pl.npad, pl.slice_n, pl.tbl_n
    tlo, thi = pl.tlo, pl.thi
    ncols_lo = int(tlo.sum()) * 8    # idx sbuf cols
    ncols_hi = int(thi.sum()) * 8

    nocc = os.environ.get("GAT_NOCC", "0") == "1"
    maxidx = int(os.environ.get("GAT_MAXIDX", "1024"))
    scratch = int(os.environ.get("GAT_SCRATCH", "16384"))
    nc = bacc.Bacc("TRN2", target_bir_lowering=False, debug=False,
                   enable_asserts=False, num_devices=num_devices,
                   dynamic_dma_scratch_size=scratch)
    h_in = nc.dram_tensor("ht", [in_dim, npad], BF16, kind="ExternalInput")
    ilo_in = nc.dram_tensor("idx_lo", [P, max(ncols_lo, 8)], I16,
                            kind="ExternalInput")
    ihi_in = nc.dram_tensor("idx_hi", [P, max(ncols_hi, 8)], I16,
                            kind="ExternalInput")

    st1_in = nc.dram_tensor("stat1", [in_dim, 136], BF16,
                            kind="ExternalInput")
    st2_in = nc.dram_tensor("stat2", [nd, out_dim + 2], F32,
                            kind="ExternalInput")
    id_in = nc.dram_tensor("ident", [P, P], BF16, kind="ExternalInput")
    idf_in = nc.dram_tensor("identf", [P, P], F32, kind="ExternalInput")
    dum_in = nc.dram_tensor("dummyrow", [2, P], F32, kind="ExternalInput")
    a1s_in = nc.dram_tensor("a1srep", [P, P], BF16, kind="ExternalInput")
    out_dram = nc.dram_tensor("out", [npad, out_dim], F32,
                              kind="ExternalOutput")

    with tile.TileContext(nc) as tc:
        with (tc.tile_pool(name="const", bufs=1) as cpool,
              tc.tile_pool(name="dram", bufs=1, space="DRAM") as dpool,
              tc.tile_pool(name="work", bufs=3) as wpool,
              tc.tile_pool(name="gath", bufs=4) as gpool,
              tc.tile_pool(name="psA", bufs=1, space="PSUM") as pspool,
              tc.tile_pool(name="psE", bufs=2, space="PSUM") as pspoolE,
              tc.tile_pool(name="psZ", bufs=3, space="PSUM") as pspoolZ,
              tc.tile_pool(name="psB", bufs=1, space="PSUM") as pspool2):
            nc.gpsimd.load_library(mlp)

            # ---- constants / persistent tiles
            # one-shot const loads on the ACT hwdge queue: the sync queue
            # reaches Z1's first hT load immediately
            ident = cpool.tile([P, P], BF16)
            nc.scalar.dma_start(ident[:], id_in[:])
            identf = cpool.tile([P, P], F32)
            nc.scalar.dma_start(identf[:], idf_in[:])
            stat1 = cpool.tile([P, kq, 136], BF16)
            nc.scalar.dma_start(
                stat1[:], st1_in.ap().rearrange("(q p) n -> p q n", p=P))
            stat2 = cpool.tile([P, out_dim + 2], F32)
            nc.scalar.dma_start(stat2[:], st2_in[:])
            idx_lo = cpool.tile([P, max(ncols_lo, 8)], I16)
            nc.scalar.dma_start(idx_lo[:], ilo_in[:])
            idx_hi = cpool.tile([P, max(ncols_hi, 8)], I16)
            nc.scalar.dma_start(idx_hi[:], ihi_in[:])
            scores1 = cpool.tile([P, chunks, 8], F32)
            scores2 = cpool.tile([P, chunks], F32)

            # es-on-the-fly (GAT_ES256): 256B table1 rows (z bf16 only);
            # es[src] is recomputed per edge from the gathered z on DVE,
            # halving AG1 and the L1 gather bytes.
            es256 = os.environ.get("GAT_ES256", "1") == "1"
            r1w = 64 if es256 else P                 # f32 words per t1 row
            slice1 = dpool.tile([slice_n, r1w], F32)
            table1 = dpool.tile([tbl_n, r1w], F32, addr_space="Shared")
            a1s = cpool.tile([P, P], BF16)
            if es256:
                nc.scalar.dma_start(a1s[:], a1s_in[:])
            # layer-2 rows are 256B (z2 bf16 + es2/ed2 f32): half the
            # AllGather traffic of layer 1
            slice2 = dpool.tile([slice_n, 64], F32)
            table2 = dpool.tile([tbl_n, 64], F32, addr_space="Shared")

            # ---- phase Z1: own rows [z|es|ed] from host-transposed bf16 h
            # batch 4 chunks per DMA so the sync engine's issue+wait chain
            # (~2us per DMA) stops pacing the phase
            hT_view = h_in.ap().rearrange("(q p) n -> p q n", p=P)
            # AG packing/splitting: move only the meaningful 288B of each
            # 512B table1 row (72 f32: z bf16 + es/ed), via a strided
            # [C, rows, 72] out view; split the AG so the first AGH chunks
            # transfer while Z1 still computes the rest.
            ZB = 3
            for k0 in range(0, chunks, ZB):
                zb = min(ZB, chunks - k0)
                hT = wpool.tile([P, kq, zb * P], BF16, tag="hT")
                nc.sync.dma_start(
                    hT[:], hT_view[:, :, k0 * P:(k0 + zb) * P])
                rowt = wpool.tile([P, zb, r1w], F32, tag="rowt")
                psz = pspoolZ.tile([P, zb, 136], F32, tag="psz")
                for j in range(zb):
                    for q in range(kq):
                        nc.tensor.matmul(psz[:, j, :],
                                         hT[:, q, j * P:(j + 1) * P],
                                         stat1[:, q, :],
                                         start=(q == 0), stop=(q == kq - 1))
                if not es256:
                    nc.vector.memset(rowt[:, :, 72:P], 0.0)
                rbf = rowt.bitcast(BF16)
                nc.vector.tensor_copy(rbf[:, :, 0:P],
                                      psz[:, :, 0:P])
                if not es256:
                    nc.scalar.copy(rowt[:, :, 64:72], psz[:, :, 128:136])
                nc.vector.tensor_copy(scores1[:, k0:k0 + zb, :],
                                      psz[:, :, 128:136])
                # issue from ACT (which produced rowt's score cols) so the
                # sync queue only paces the hT loads
                nc.scalar.dma_start(
                    slice1[k0 * P:(k0 + zb) * P, :]
                    .rearrange("(c p) n -> p c n", p=P),
                    rowt[:])
            nc.sync.dma_start(slice1[npad:npad + 1, :], dum_in[0:1, 0:r1w])
            if nocc:
                pass
            else:
                nc.gpsimd.collective_compute(
                    "AllGather", ALU.bypass,
                    replica_groups=[list(range(num_devices))],
                    ins=[slice1[0:slice_n, :].opt()],
                    outs=[table1[0:tbl_n, :].opt()])

            elvl = int(os.environ.get("GAT_ELVL", "9"))

            # ---- edge phases
            def edge_phase(layer, table, scores_t, after_group=None):
                pay_w = P if layer == 1 else out_dim     # payload cols
                nh = 4 if layer == 1 else 1              # heads
                mw = pay_w + nh                          # payload + ex cols
                # gathered bf16/row: L1 rows are z-only under es256
                elem = (128 if es256 else 256) if layer == 1 else 128
                tbl_bf = table.bitcast(BF16)
                if getattr(pl, "signed_grid", False):
                    # signed idx: AP based at row SBASE, idx = pos - SBASE
                    lo_src = tbl_bf[SBASE:tbl_n, :]
                    hi_src = lo_src                      # thi == 0, unused
                else:
                    lo_src = tbl_bf[0:pl.hi_base, :]
                    hi_src = tbl_bf[pl.hi_base:tbl_n, :]
                for grp in pl.groups:
                    nlo = int(tlo[grp].sum())
                    nhi = int(thi[grp].sum())
                    gt = gpool.tile([P, nlo + nhi, elem], BF16, tag="gt")
                    mt = maxidx // P
                    for (src_ap, idxt, base_t, n_t, dst0) in (
                            (lo_src, idx_lo, int(pl.lo_tile_base[grp[0]]),
                             nlo, 0),
                            (hi_src, idx_hi, int(pl.hi_tile_base[grp[0]]),
                             nhi, nlo)):
                        done = 0
                        while done < n_t:
                            nt = min(n_t - done, mt)
                            c0 = (base_t + done) * 8
                            nc.gpsimd.dma_gather(
                                gt[:, dst0 + done:dst0 + done + nt, :],
                                src_ap, idxt[:, c0:c0 + nt * 8],
                                nt * P, nt * P, elem)
                            done += nt
                    gt32 = gt.bitcast(F32)
                    lo_b = int(pl.lo_tile_base[grp[0]])
                    hi_b = int(pl.hi_tile_base[grp[0]])
                    if elvl == 0:
                        sink = wpool.tile([P, 64], F32, tag="sink")
                        nc.vector.tensor_copy(sink[:], gt32[:, 0, 0:64])
                        nc.sync.dma_start(
                            slice2[grp[0] * P:(grp[0] + 1) * P, :], sink[:])
                        continue
                    for k in grp:
                        tl, th = int(tlo[k]), int(thi[k])
                        T = tl + th
                        ko_lo = int(pl.lo_tile_base[k]) - lo_b
                        ko_hi = nlo + int(pl.hi_tile_base[k]) - hi_b
                        # e = es[src] + ed[dst]
                        e32 = wpool.tile([P, T, nh], F32, tag="e32")
                        for (off, cnt, eo) in ((ko_lo, tl, 0), (ko_hi, th, tl)):
                            if cnt == 0:
                                continue
                            if layer == 1 and es256:
                                # es[src] = sum_j z[h*32+j]*a1s[h*32+j],
                                # recomputed from the gathered bf16 z
                                prod = wpool.tile([P, cnt, 4, 32], BF16,
                                                  tag="prod")
                                nc.vector.tensor_tensor(
                                    prod[:],
                                    gt[:, off:off + cnt, :].rearrange(
                                        "p t (a b) -> p t a b", a=4),
                                    a1s[:].rearrange(
                                        "p (a b) -> p a b", a=4)
                                    .unsqueeze(1)
                                    .broadcast_to([P, cnt, 4, 32]),
                                    ALU.mult)
                                esvt = wpool.tile([P, cnt, 4], F32,
                                                  tag="esvt")
                                nc.vector.tensor_reduce(
                                    esvt[:], prod[:],
                                    mybir.AxisListType.X, ALU.add)
                                esv = esvt[:]
                                edv = (scores_t[:, k, 4:8].unsqueeze(1)
                                       .broadcast_to([P, cnt, 4]))
                            elif layer == 1:
                                esv = gt32[:, off:off + cnt, 64:68]
                                edv = (scores_t[:, k, 4:8].unsqueeze(1)
                                       .broadcast_to([P, cnt, 4]))
                            else:
                                esv = gt32[:, off:off + cnt, 32:33]
                                edv = (scores_t[:, k:k + 1].unsqueeze(1)
                                       .broadcast_to([P, cnt, 1]))
                            nc.vector.tensor_tensor(
                                e32[:, eo:eo + cnt, :], esv, edv, ALU.add)
                        ef = e32[:].rearrange("p t h -> p (t h)")
                        lr = wpool.tile([P, T, nh], F32, tag="lr")
                        lrf = lr[:].rearrange("p t h -> p (t h)")
                        nc.vector.scalar_tensor_tensor(
                            lrf, ef, 0.01, ef, ALU.mult, ALU.max)
                        # payex: [pay | ex] so one matmul accumulates the
                        # weighted sum and the softmax denominator together
                        payex = wpool.tile([P, T, mw], BF16, tag="payex")
                        nc.scalar.activation(
                            payex[:, 0:T, pay_w:mw], lr[:], AF.Exp)
                        if elvl == 1:
                            sink = wpool.tile([P, 64], F32, tag="sink")
                            nc.vector.memset(sink[:], 0.0)
                            nc.sync.dma_start(
                                slice2[k * P:(k + 1) * P, :], sink[:])
                            continue
                        psz = pspoolE.tile([P, mw], F32, tag="psE")
                        # two passes: all DVE multiplies first, then all
                        # matmuls — keeps DVE of chunk k+1 overlapping the
                        # TensorE accumulation of chunk k
                        for t in range(T):
                            col = (ko_lo + t) if t < tl else (ko_hi + t - tl)
                            if layer == 1:
                                zin = gt[:, col, 0:P].rearrange(
                                    "p (a b) -> p a b", a=4)
                                exv = (payex[:, t, pay_w:mw].unsqueeze(2)
                                       .broadcast_to([P, 4, 32]))
                                nc.vector.tensor_tensor(
                                    payex[:, t, 0:pay_w].rearrange(
                                        "p (a b) -> p a b", a=4),
                                    zin, exv, ALU.mult)
                            else:
                                zin = gt[:, col, 0:out_dim]
                                exv = (payex[:, t, pay_w:mw]
                                       .broadcast_to([P, out_dim]))
                                nc.vector.tensor_tensor(
                                    payex[:, t, 0:pay_w], zin, exv, ALU.mult)
                        for t in range(T):
                            nc.tensor.matmul(psz[:], ident[:], payex[:, t, :],
                                             start=(t == 0), stop=(t == T - 1))
                        # epilogue
                        den = wpool.tile([P, nh], F32, tag="den")
                        nc.vector.tensor_scalar_add(den[:], psz[:, pay_w:mw],
                                                    1e-30)
                        rec = wpool.tile([P, nh], F32, tag="rec")
                        nc.vector.reciprocal(rec[:], den[:])
                        if layer == 1:
                            h1 = wpool.tile([P, P], F32, tag="h1")
                            rv = (rec[:].unsqueeze(2)
                                  .broadcast_to([P, 4, 32]))
                            nc.vector.tensor_tensor(
                                h1[:].rearrange("p (a b) -> p a b", a=4),
                                psz[:, 0:P].rearrange("p (a b) -> p a b", a=4),
                                rv, ALU.mult)
                            # elu(x) = max(x, exp(min(x,0)) - 1)
                            mn = wpool.tile([P, P], F32, tag="mn")
                            nc.vector.tensor_scalar_min(mn[:], h1[:], 0.0)
                            em = wpool.tile([P, P], F32, tag="em")
                            nc.scalar.activation(em[:], mn[:], AF.Exp)
                            h1e = wpool.tile([P, P], F32, tag="h1e")
                            nc.vector.scalar_tensor_tensor(
                                h1e[:], em[:], -1.0, h1[:], ALU.add, ALU.max)
                            # z2 = h1e @ [W2 | W2 a2s | W2 a2d]
                            pst2 = pspool.tile([P, P], F32, tag="pst2")
                            nc.tensor.matmul(pst2[:], h1e[:], identf[:],
                                             is_transpose=True)
                            h1T = wpool.tile([P, P], F32, tag="h1T")
                            nc.vector.tensor_copy(h1T[:], pst2[:])
                            psz2 = pspool2.tile([P, out_dim + 2], F32,
                                                tag="psz2")
                            nc.tensor.matmul(psz2[:], h1T[:], stat2[:])
                            # layer-2 row: z2 as bf16 in bytes [0,128),
                            # es2/ed2 f32 at f32 cols 32:34 (256B rows)
                            row2 = wpool.tile([P, 64], F32, tag="row2")
                            r2bf = row2.bitcast(BF16)
                            nc.vector.tensor_copy(r2bf[:, 0:out_dim],
                                                  psz2[:, 0:out_dim])
                            nc.scalar.copy(row2[:, 32:34],
                                           psz2[:, out_dim:out_dim + 2])
                            nc.vector.tensor_copy(scores2[:, k:k + 1],
                                                  psz2[:, out_dim + 1:
                                                       out_dim + 2])
                            nc.sync.dma_start(slice2[k * P:(k + 1) * P, 0:34],
                                              row2[:, 0:34])
                        else:
                            orow = wpool.tile([P, out_dim], F32, tag="orow")
                            rv = rec[:].broadcast_to([P, out_dim])
                            nc.vector.tensor_tensor(orow[:], psz[:, 0:out_dim],
                                                    rv, ALU.mult)
                            nc.sync.dma_start(
                                out_dram[k * P:(k + 1) * P, :], orow[:])
                    if after_group is not None:
                        after_group(grp)

            if phase == "z1":
                nc.sync.dma_start(out_dram[0:npad, :],
                                  slice1[0:npad, 0:out_dim])
            elif phase == "e1":
                edge_phase(1, table1, scores1)
                nc.sync.dma_start(out_dram[0:npad, :],
                                  slice2[0:npad, 0:out_dim])
            else:
                # constant dummy row: write before the edge phase so AG2
                # never waits on a late tiny DMA
                nc.sync.dma_start(slice2[npad:npad + 1, :], dum_in[1:2, 0:64])
                edge_phase(1, table1, scores1)
                if nocc:
                    pass
                else:
                    nc.gpsimd.collective_compute(
                        "AllGather", ALU.bypass,
                        replica_groups=[list(range(num_devices))],
                        ins=[slice2[0:slice_n, :].opt()],
                        outs=[table2[0:tbl_n, :].opt()])
                edge_phase(2, table2, scores2)

    nc.compile()
    return nc


# --------------------------------------------------------------------------
# host wrapper
# --------------------------------------------------------------------------

def make_inputs(pl, h, W1, a1, W2, a2, in_dim, hid, heads, out_dim):
    n_nodes = h.shape[0]
    npc, npad = pl.npc, pl.npad
    nd = heads * hid
    # stationaries
    st1 = np.zeros((in_dim, 136), np.float32)
    for hh in range(heads):
        st1[:, hh * hid:(hh + 1) * hid] = W1[hh]
        st1[:, 128 + hh] = W1[hh] @ a1[hh, :hid]
        st1[:, 132 + hh] = W1[hh] @ a1[hh, hid:]
    st2 = np.zeros((nd, out_dim + 2), np.float32)
    st2[:, :out_dim] = W2[0]
    st2[:, out_dim] = W2[0] @ a2[0, :out_dim]
    st2[:, out_dim + 1] = W2[0] @ a2[0, out_dim:]
    import os
    es256 = os.environ.get("GAT_ES256", "1") == "1"
    dummy = np.zeros((2, 128), np.float32)
    if es256:
        # L1 dummy z: per head h, zd block = -1e5 * a1h / ||a1h||^2 so the
        # recomputed es_h(dummy) = -1e5 for every head -> ex = exp(...) = 0
        zd = np.zeros(128, np.float32)
        for hh in range(heads):
            ah = a1[hh, :hid].astype(np.float64)
            zd[hh * hid:(hh + 1) * hid] = (-1e5 / float(ah @ ah)) * ah
        dummy[0, 0:64] = zd.astype(BF).view(np.float32)
    else:
        dummy[0, 64:72] = -1e30
    dummy[1, 32] = -1e30   # layer-2 dummy row: es2 slot (256B rows)
    a1svec = np.concatenate([a1[hh, :hid] for hh in range(heads)])
    a1srep = np.tile(a1svec.astype(BF)[None, :], (128, 1))
    identb = np.eye(128, dtype=BF)
    identf = np.eye(128, dtype=np.float32)

    ncols_lo = int(pl.tlo.sum()) * 8
    ncols_hi = int(pl.thi.sum()) * 8
    in_maps = []
    st1b = st1.astype(BF)
    for c in range(C):
        hp = np.zeros((npad, in_dim), np.float32)
        valid = pl.perm[c] < npc
        hp[valid] = h[c * npc + pl.perm[c][valid]]
        ht = np.ascontiguousarray(hp.T).astype(BF)
        ilo = wrap_idx(pl.idx_lo[c]) if ncols_lo else \
            np.zeros((128, 8), np.int16)
        ihi = wrap_idx(pl.idx_hi[c]) if ncols_hi else \
            np.zeros((128, 8), np.int16)
        in_maps.append({
            "ht": ht, "idx_lo": ilo, "idx_hi": ihi,
            "stat1": st1b, "stat2": st2, "ident": identb, "identf": identf,
            "dummyrow": dummy, "a1srep": a1srep,
        })
    return in_maps


def unpermute(pl, outs, n_nodes, out_dim):
    npc = pl.npc
    full = np.zeros((n_nodes, out_dim), np.float32)
    for c in range(C):
        valid = pl.perm[c] < npc
        full[c * npc + pl.perm[c][valid]] = outs[c][valid]
    return full


def gat_run(h, src, dst, W1, a1, W2, a2, runner):
    """Full pipeline; `runner(nc, in_maps)` -> list of per-core {'out': arr}."""
    import os
    n_nodes, in_dim = h.shape
    heads, _, hid = W1.shape
    out_dim = W2.shape[2]
    if os.environ.get("GAT_SG", "1") == "1":
        pl = build_plan_sg(np.asarray(src), np.asarray(dst), n_nodes)
    else:
        pl = build_plan(np.asarray(src), np.asarray(dst), n_nodes)
    nc = build_program(pl, in_dim, hid, heads, out_dim)
    in_maps = make_inputs(pl, np.asarray(h, np.float32), np.asarray(W1),
                          np.asarray(a1), np.asarray(W2), np.asarray(a2),
                          in_dim, hid, heads, out_dim)
    outs = runner(nc, in_maps)
    return unpermute(pl, [o["out"] for o in outs], n_nodes, out_dim)


def hw_runner(nc, in_maps, trace=None):
    import os
    from concourse.bass_utils import run_bass_kernel_spmd
    if trace is None:
        trace = os.environ.get("GAT_TRACE", "0") == "1"
    res = run_bass_kernel_spmd(nc, in_maps, core_ids=list(range(C)),
                               trace=trace)
    hw_runner.last = res
    return res.results


def kernel(**inputs):
    out = gat_run(inputs["h"], inputs["src"], inputs["dst"], inputs["W1"],
                  inputs["a1"], inputs["W2"], inputs["a2"], hw_runner)
    return out

